# revision 70
# baseline (speedup 1.0000x reference)
"""DNeRF renderer on 8 Trainium2 cores (Bass/Tile).

Data-parallel over rays (1024 rays/core, 8 ray-tiles of 128 rays).
Per-ray sort/searchsorted machinery uses closed-form merge ranks +
GPSIMD local_scatter + tensor_tensor_scan.  MLPs run on the PE in a
transposed pair layout (HBM bridge) with float32r moving operands
(4x PE throughput at fp32-class precision).  The per-tile work is
software-pipelined in three phases -- coarse MLP (k), sampling chain
(k-1), fine MLP + composite (k-2) -- so the serial sampling chain of
one tile overlaps the MLP phases of its neighbours.  Transmittance
cumprods are telescoped (w_i = T_i - T_{i+1}) to shorten the chains,
and the per-ray color d-term is folded into the PE via an indicator
matmul accumulation instead of a DVE broadcast add.
"""

import numpy as np
from contextlib import ExitStack

import concourse.bass as bass
import concourse.bacc as bacc
import concourse.mybir as mybir
import concourse.tile as tile
from concourse.bass_utils import run_bass_kernel_spmd
from concourse import library_config

dt = mybir.dt
Alu = mybir.AluOpType
Act = mybir.ActivationFunctionType
AxX = mybir.AxisListType.X

NCORES = 8
NRAYS = 8192
R = NRAYS // NCORES      # rays per core
P = 128                  # rays per tile (partitions)
T = R // P               # ray-tiles per core
S = 64                   # num_steps
U = 64                   # upsample_steps
M = S + U                # merged samples
MIN_NEAR = 0.05
DS = 1.0                 # DENSITY_SCALE
M24 = 16777216.0         # 2^24

_BUILT = None


def _build():
    nc = bacc.Bacc("TRN2", target_bir_lowering=False, debug=False,
                   num_devices=NCORES)

    def din(name, shape):
        return nc.dram_tensor(name, shape, dt.float32, kind="ExternalInput").ap()

    rays_o = din("rays_o_k", [P, T, 3])
    rays_d = din("rays_d_k", [P, T, 3])
    dT_in = din("dT_k", [T, 4, P])
    v128 = din("v128", [P, S])
    iotap1_r = din("iotap1_r", [P, M])
    iotaev62 = din("iotaev62", [P, 62])
    iota62e5 = din("iota62e5", [P, 62])
    zero128 = din("zero128", [P, M])
    cc = din("cc", [P, 12])
    lhsT6 = din("lhsT6", [6, 128])
    b1rep = din("b1rep", [128, 1])
    w0pair = din("w0pair", [128, 2])
    wgcpair = din("wgcpair", [128, 128])
    wc2pair = din("wc2pair", [128, 6])
    dlhs = din("dlhs", [4, 64])
    ind16 = din("ind16", [16, 2048])
    bgrep = din("bgrep", [P, 3])
    bc2p6 = din("bc2p6", [6, 1])
    scl_in = din("scl", [P, 4])

    img_out = nc.dram_tensor("img_k", [P, T, 3], dt.float32,
                             kind="ExternalOutput").ap()

    fh = P * M // 2   # 8192 fine pair-cols per tile
    half = P * S // 2  # 4096 coarse pair-cols per tile

    with tile.TileContext(nc) as tc, ExitStack() as ctx:

        cpool = ctx.enter_context(tc.tile_pool(name="consts", bufs=1))
        spool = ctx.enter_context(tc.tile_pool(name="setup", bufs=1))
        wpool = ctx.enter_context(tc.tile_pool(name="work", bufs=2))
        gpool = ctx.enter_context(tc.tile_pool(name="geo", bufs=T))
        mpool = ctx.enter_context(tc.tile_pool(name="mlp", bufs=3))
        ppool = ctx.enter_context(tc.tile_pool(name="psum", bufs=2, space="PSUM"))
        dpool = ctx.enter_context(tc.tile_pool(name="dram", bufs=2, space="DRAM"))

        def cload(ap_in, shape, tag, dtype=dt.float32):
            t_ = cpool.tile(shape, dtype, tag=tag, name=tag)
            nc.sync.dma_start(t_[:], ap_in)
            return t_

        def cloadr(ap_in, shape, tag):
            # f32r-typed const (matmul operand; producers must emit f32r)
            t_ = cpool.tile(shape, dt.float32r, tag=tag, name=tag)
            nc.sync.dma_start(t_[:], ap_in.bitcast(dt.float32r))
            return t_

        v128_s = cload(v128, [P, S], tag='c_v128')
        iop1_s = cload(iotap1_r, [P, M], tag='c_iotap1_r')
        iev62_s = cload(iotaev62, [P, 62], tag='c_iotaev62')
        ie5_s = cload(iota62e5, [P, 62], tag='c_iota62e5')
        zero_s = cload(zero128, [P, M], tag='c_zero128')
        cc_s = cload(cc, [P, 12], tag='c_cc')
        lhsT6_s = cloadr(lhsT6, [6, 128], tag='c_lhsT6')
        b1_s = cload(b1rep, [128, 1], tag='c_b1rep')
        w0p_s = cloadr(w0pair, [128, 2], tag='c_w0pair')
        wgc_s = cloadr(wgcpair, [128, 128], tag='c_wgcpair')
        wc2_s = cloadr(wc2pair, [128, 6], tag='c_wc2pair')
        dlhs_s = cload(dlhs, [4, 64], tag='c_dlhs')
        ind16_s = cloadr(ind16, [16, 2048], tag='c_ind16')
        bg_s = cload(bgrep, [P, 3], tag='c_bgrep')
        bc2_s = cload(bc2p6, [6, 1], tag='c_bc2p6')
        scl_s = cload(scl_in, [P, 4], tag='c_scl_in')
        ro_s = cload(rays_o, [P, T, 3], tag='c_rays_o')
        rd_s = cload(rays_d, [P, T, 3], tag='c_rays_d')

        def bc(col, n):
            return col.broadcast_to((P, n))

        ones_c = cc_s[:, 0:1]
        neg1_c = cc_s[:, 1:2]
        eps_c = cc_s[:, 2:3]
        nhalf_c = cc_s[:, 5:6]
        nm24_c = cc_s[:, 6:7]
        n2_c = cc_s[:, 7:8]
        m24_c = cc_s[:, 8:9]
        bd2_0c = scl_s[:, 0:1]

        # ================= STAGE A: ray setup (all tiles) ================
        n24 = T * 3

        def st(shape, tag, dtype=dt.float32):
            return spool.tile(shape, dtype, tag=tag, name=tag)

        negd = st([P, T, 3], 's_negd')
        nc.vector.tensor_scalar(negd[:], rd_s[:], -1.0, None, Alu.mult)
        absd = st([P, T, 3], 's_absd')
        nc.vector.tensor_tensor(absd[:], rd_s[:], negd[:], Alu.max)
        dmask = st([P, T, 3], 's_dmask', dt.uint8)
        nc.vector.tensor_scalar(dmask[:], absd[:], 1e-9, None, Alu.is_lt)
        dsafe = st([P, T, 3], 's_dsafe')
        nc.vector.select(dsafe[:].rearrange("p t c -> p (t c)"),
                         dmask[:].rearrange("p t c -> p (t c)"),
                         bc(eps_c, n24),
                         rd_s[:].rearrange("p t c -> p (t c)"))
        invd = st([P, T, 3], 's_invd')
        nc.vector.reciprocal(invd[:], dsafe[:])
        a1 = st([P, T, 3], 's_a1')
        nc.vector.scalar_tensor_tensor(a1[:], ro_s[:], 1.0, invd[:],
                                       Alu.add, Alu.mult)
        b1 = st([P, T, 3], 's_b1')
        nc.vector.scalar_tensor_tensor(b1[:], ro_s[:], -1.0, invd[:],
                                       Alu.add, Alu.mult)
        mx = st([P, T, 3], 's_mx')
        nc.vector.tensor_tensor(mx[:], a1[:], b1[:], Alu.max)
        mn = st([P, T, 3], 's_mn')
        nc.vector.tensor_tensor(mn[:], a1[:], b1[:], Alu.min)
        tmin = st([P, T], 's_tmin')
        nc.vector.tensor_reduce(tmin[:], mx[:], AxX, Alu.min)
        tmax = st([P, T], 's_tmax')
        nc.vector.tensor_reduce(tmax[:], mn[:], AxX, Alu.max)
        near = st([P, T], 's_near')
        nc.vector.tensor_scalar(near[:], tmin[:], -1.0, MIN_NEAR,
                                Alu.mult, Alu.max)
        tmaxt = st([P, T], 's_tmaxt')
        nc.vector.tensor_scalar(tmaxt[:], tmax[:], -1.0, None, Alu.mult)
        fmask = st([P, T], 's_fmask', dt.uint8)
        nc.vector.tensor_tensor(fmask[:], tmaxt[:], near[:], Alu.is_lt)
        nearp = st([P, T], 's_nearp')
        nc.vector.tensor_scalar(nearp[:], near[:], 1e-2, None, Alu.add)
        far = st([P, T], 's_far')
        nc.vector.select(far[:], fmask[:], nearp[:], tmaxt[:])
        rng = st([P, T], 's_rng')
        nc.vector.tensor_tensor(rng[:], far[:], near[:], Alu.subtract)
        dzv = st([P, T], 's_dzv')
        nc.vector.tensor_scalar(dzv[:], rng[:], 1.0 / 63.0, None, Alu.mult)
        sdv = st([P, T], 's_sdv')
        nc.vector.tensor_scalar(sdv[:], rng[:], 1.0 / 64.0, None, Alu.mult)
        invdz = st([P, T], 's_invdz')
        nc.vector.reciprocal(invdz[:], dzv[:])
        inv2dz = st([P, T], 's_inv2dz')
        nc.vector.tensor_scalar(inv2dz[:], invdz[:], 2.0, None, Alu.mult)
        mid0 = st([P, T], 's_mid0')
        nc.vector.scalar_tensor_tensor(mid0[:], dzv[:], 0.5, near[:],
                                       Alu.mult, Alu.add)
        aoff = st([P, T], 's_aoff')
        nc.vector.tensor_tensor(aoff[:], near[:], dzv[:], Alu.subtract)

        def wt(shape, tag, dtype=dt.float32):
            return wpool.tile(shape, dtype, tag=tag, name=tag)

        def mt(shape, tag, dtype=dt.float32):
            return mpool.tile(shape, dtype, tag=tag, name=tag)

        state = [dict() for _ in range(T)]

        scr_all = dpool.tile([T, 2, 3, 64, S], dt.float32, tag="xyzscr",
                             name="xyzscr")
        sgs_all = dpool.tile([T, 2, 64, S], dt.float32, tag="sigscr",
                             name="sigscr")
        scr2_all = dpool.tile([T, 2, 3, 64, M], dt.float32, tag="xyzscr2",
                              name="xyzscr2")
        sgs2_all = dpool.tile([T, 2, 64, M], dt.float32, tag="sigscr2",
                              name="sigscr2")
        rgbs_all = dpool.tile([T, 2, 3, 64, M], dt.float32, tag="rgbscr",
                              name="rgbscr")
        dtm_all = spool.tile([16, T * 512], dt.float32r, tag="dtmT4",
                             name="dtmT4")

        # --- pre-emit per-tile geometry + color d-term (stage-A deps only;
        # keeps the per-iteration DVE queue from gating next tiles' MLPs) ---
        for t in range(T):
            near_c = near[:, t:t + 1]
            rng_c = rng[:, t:t + 1]
            sd_c = sdv[:, t:t + 1]
            zc = gpool.tile([P, S], dt.float32, tag="zc", name="zc")
            nc.vector.scalar_tensor_tensor(zc[:], v128_s[:], rng_c,
                                           bc(near_c, S), Alu.mult, Alu.add)
            deltas = gpool.tile([P, S], dt.float32, tag="deltas",
                                name="deltas")
            nc.vector.tensor_tensor(deltas[:, 0:S - 1], zc[:, 1:S],
                                    zc[:, 0:S - 1], Alu.subtract)
            nc.scalar.copy(deltas[:, S - 1:S], sd_c)
            state[t]['deltas'] = deltas
            xyzc = gpool.tile([P, 3, S], dt.float32, tag="xyzc", name="xyzc")
            for c in range(3):
                nc.vector.scalar_tensor_tensor(
                    xyzc[:, c, :], zc[:], rd_s[:, t, c:c + 1],
                    bc(ro_s[:, t, c:c + 1], S), Alu.mult, Alu.add)
            nc.vector.scalar_tensor_tensor(
                xyzc[:].rearrange("p c s -> p (c s)"),
                xyzc[:].rearrange("p c s -> p (c s)"),
                1.0, bc(neg1_c, 3 * S), Alu.min, Alu.max)
            for hh in range(2):
                nc.sync.dma_start(
                    scr_all[t, hh].rearrange("c pl s -> pl c s"),
                    xyzc[64 * hh:64 * (hh + 1)])

            dT_sb = gpool.tile([4, P], dt.float32, tag="dTsb", name="dTsb")
            nc.sync.dma_start(dT_sb[:], dT_in[t])
            pDT = ppool.tile([16, 512], dt.float32, tag="pG", name="ppG")
            for g in range(4):
                for hh in range(2):
                    nc.tensor.matmul(
                        pDT[:, g * 128 + hh * 64:g * 128 + (hh + 1) * 64],
                        dT_sb[:, hh * 64 + g * 16:hh * 64 + (g + 1) * 16],
                        dlhs_s[:], start=True, stop=True)
            nc.vector.tensor_copy(dtm_all[:, t * 512:(t + 1) * 512], pDT[:])

        # ---------------- phase C: coarse z/xyz + density MLP ------------
        # ---- continuous 1024-col group pipeline across tiles: stage
        # queues persist so there is no per-tile fill/drain ----
        NGc = 4
        NGf = 8
        coarseA_q = []   # (t, G, rh1g)
        fineA_q = []     # (t, G, rh1g)
        fineC_q = []     # (t, G, ch1g)
        coarse_hold = {}
        fine_hold = {}

        def stageA(t, g, scrv):
            gg, h2 = divmod(g, 2)
            hold = coarse_hold if scrv is None else fine_hold
            if h2 == 0:
                view = state[t]['scrv'] if scrv is None else scrv
                rhs2k = mt([6, 2048], "rhs6", dt.float32r)
                nc.sync.dma_start(rhs2k[:],
                                  view[:, gg * 2048:(gg + 1) * 2048])
                hold['rhs'] = rhs2k
            rhs2k = hold['rhs']
            pA = ppool.tile([128, 1024], dt.float32, tag="pA", name="ppA")
            for c in range(2):
                nc.tensor.matmul(
                    pA[:, 512 * c:512 * (c + 1)], lhsT6_s[:],
                    rhs2k[:, h2 * 1024 + 512 * c:h2 * 1024 + 512 * (c + 1)],
                    start=True, stop=True)
            rh1g = mt([128, 1024], "rh1", dt.float32r)
            if g % 2 == 0:
                nc.vector.tensor_scalar(rh1g[:], pA[:], b1_s[:, 0:1],
                                        0.0, Alu.add, Alu.max)
            else:
                nc.scalar.activation(rh1g[:], pA[:], Act.Relu,
                                     bias=b1_s[:, 0:1])
            return rh1g

        def sigma_stage(t, G, rh1g, hold, sgsf_key, nper):
            gg, h2 = divmod(G, 2)
            pS = ppool.tile([2, 1024], dt.float32, tag="pA", name="ppA")
            for c in range(2):
                nc.tensor.matmul(pS[:, 512 * c:512 * (c + 1)],
                                 w0p_s[:, 0:2],
                                 rh1g[:, 512 * c:512 * (c + 1)],
                                 start=True, stop=True)
            if h2 == 0:
                hold['sg'] = mt([2, 2048], "sgsb")
            sg_sb = hold['sg']
            if G % 2 == 0:
                nc.vector.tensor_copy(
                    sg_sb[:, h2 * 1024:(h2 + 1) * 1024], pS[:])
            else:
                nc.scalar.copy(
                    sg_sb[:, h2 * 1024:(h2 + 1) * 1024], pS[:])
            if h2 == 1:
                nc.sync.dma_start(
                    state[t][sgsf_key][:, gg * 2048:(gg + 1) * 2048],
                    sg_sb[:])

        def coarseB(t, G, rh1g):
            sigma_stage(t, G, rh1g, coarse_hold, 'sgsf', NGc)
            if G == NGc - 1:
                h20 = wt([P, S], "h20")
                nc.sync.dma_start(
                    h20[:], sgs_all[t].rearrange("h p s -> (h p) s"))
                state[t]['h20'] = h20

        def emit_coarse(t):
            stt = state[t]
            stt['scrv'] = scr_all[t].rearrange("h c pl s -> (h c) (pl s)") \
                .bitcast(dt.float32r)
            stt['sgsf'] = sgs_all[t].rearrange("h p s -> h (p s)")
            for g in range(NGc):
                yield
                coarseA_q.append((t, g, stageA(t, g, None)))
                if len(coarseA_q) > 2:
                    coarseB(*coarseA_q.pop(0))

        def emit_coarse_drain():
            while coarseA_q:
                yield
                coarseB(*coarseA_q.pop(0))

        # -------- phase S: composite -> pdf -> scatter -> merge ----------
        def emit_sampling(t):
            stt = state[t]
            near_c = near[:, t:t + 1]
            dz_c = dzv[:, t:t + 1]
            sd_c = sdv[:, t:t + 1]
            i2dz_c = inv2dz[:, t:t + 1]
            mid0_c = mid0[:, t:t + 1]
            aoff_c = aoff[:, t:t + 1]
            h20 = stt['h20']
            deltas = stt['deltas']

            # sigma, transmittance (telescoped: Tc_i = prod_{j<=i} em_j)
            sig = wt([P, S], "sig")
            nc.scalar.activation(sig[:], h20[:], Act.Exp, bias=bd2_0c)
            dsg = wt([P, S], "dsg")
            nc.vector.tensor_tensor(dsg[:], deltas[:], sig[:], Alu.mult)
            em = wt([P, S], "em")
            nc.scalar.activation(em[:], dsg[:], Act.Exp, scale=-DS)
            Tc = wt([P, S], "Tcz")
            nc.vector.tensor_tensor_scan(Tc[:], em[:], zero_s[:, 0:S],
                                         1.0, Alu.mult, Alu.add)
            # cdfraw_j = Tc_0 - Tc_{j+1} + (j+1)*1e-5  (j = 0..61)
            cdfraw = wt([P, 62], "cdfraw")
            nc.vector.scalar_tensor_tensor(cdfraw[:], Tc[:, 1:63], -1.0,
                                           bc(Tc[:, 0:1], 62),
                                           Alu.mult, Alu.add)
            nc.vector.tensor_tensor(cdfraw[:], cdfraw[:], ie5_s[:], Alu.add)
            pinv = wt([P, 1], "pinv")
            nc.vector.reciprocal(pinv[:], cdfraw[:, 61:62])
            cdf = wt([P, 62], "cdf")
            nc.vector.tensor_scalar(cdf[:], cdfraw[:], pinv[:], None, Alu.mult)

            # ---- scatter cdf onto 128-slot (cdf U u) timeline ----
            r2 = wt([P, 62], "r2")
            nc.scalar.activation(r2[:], cdf[:], Act.Identity,
                                 scale=128.0, bias=m24_c)
            nc.scalar.activation(r2[:], r2[:], Act.Identity, bias=nm24_c)
            idx2f = wt([P, 124], "idx2f")
            ev = idx2f[:].rearrange("p (a b) -> p a b", b=2)[:, :, 0:1] \
                .rearrange("p a b -> p (a b)")
            od = idx2f[:].rearrange("p (a b) -> p a b", b=2)[:, :, 1:2] \
                .rearrange("p a b -> p (a b)")
            nc.vector.tensor_tensor(ev, r2[:], iev62_s[:], Alu.add)
            nc.vector.scalar_tensor_tensor(od, r2[:], 1.0, iev62_s[:],
                                           Alu.add, Alu.add)
            idx2i = wt([P, 124], "idx2i", dt.int16)
            nc.gpsimd.tensor_copy(idx2i[:], idx2f[:])
            tlc2 = wt([P, 256], "tlc2", dt.int16)
            nc.gpsimd.local_scatter(tlc2[:], cdf[:].bitcast(dt.int16),
                                    idx2i[:], channels=P, num_elems=256,
                                    num_idxs=124)
            tlc = tlc2[:].bitcast(dt.float32)

            # ---- fills and counts on the timeline ----
            notC = wt([P, M], "notC")
            nc.vector.tensor_scalar(notC[:], tlc, 0.0, None, Alu.is_equal)
            kp1 = wt([P, M], "kp1")
            nc.vector.tensor_tensor_scan(kp1[:], notC[:], zero_s[:],
                                         0.0, Alu.add, Alu.add)
            uu = wt([P, M], "uu")
            nc.gpsimd.tensor_scalar(uu[:], kp1[:], 1.0 / 64.0, -1.0 / 128.0,
                                    Alu.mult, Alu.add)
            cntC = wt([P, M], "cntC")
            nc.vector.tensor_tensor(cntC[:], iop1_s[:], kp1[:], Alu.subtract)
            ffwd = wt([P, M], "ffwd")
            nc.vector.tensor_tensor_scan(ffwd[:], notC[:], tlc, 0.0,
                                         Alu.mult, Alu.add)
            rbwd = wt([P, M], "rbwd")
            nc.vector.tensor_tensor_scan(rbwd[:], notC[:, ::-1],
                                         tlc[:, ::-1], 0.0,
                                         Alu.mult, Alu.add)
            bwd = rbwd[:, ::-1]

            # ---- inverse-CDF lerp at u slots ----
            den = wt([P, M], "den")
            nc.vector.tensor_tensor(den[:], bwd, ffwd[:], Alu.subtract)
            nc.vector.tensor_scalar(den[:], den[:], 1e-5, None, Alu.max)
            rden = wt([P, M], "rden")
            nc.vector.reciprocal(rden[:], den[:])
            tt = wt([P, M], "tt")
            nc.vector.tensor_tensor(tt[:], uu[:], ffwd[:], Alu.subtract)
            nc.vector.tensor_tensor(tt[:], tt[:], rden[:], Alu.mult)
            bg0 = wt([P, M], "bg0")
            nc.scalar.activation(bg0[:], cntC[:], Act.Identity,
                                 scale=dz_c, bias=mid0_c)
            nz = wt([P, M], "nz")
            nc.vector.scalar_tensor_tensor(nz[:], tt[:], dz_c, bg0[:],
                                           Alu.mult, Alu.add)

            # ---- merge ranks into final (coarse U fine) timeline ----
            q2 = wt([P, M], "q2")
            nc.vector.scalar_tensor_tensor(q2[:], nz[:], near_c,
                                           bc(i2dz_c, M), Alu.subtract,
                                           Alu.mult)
            nc.vector.tensor_scalar(q2[:], q2[:], 1.0, M24, Alu.add, Alu.add)
            nc.vector.tensor_scalar(q2[:], q2[:], M24, 0.0, Alu.subtract,
                                    Alu.max)
            nc.vector.tensor_scalar(q2[:], q2[:], 126.0, None, Alu.min)
            tk2 = wt([P, M], "tk2")
            nc.gpsimd.tensor_scalar(tk2[:], kp1[:], 2.0, -2.0,
                                    Alu.mult, Alu.add)
            mk2 = wt([P, M], "mk2")
            nc.vector.tensor_scalar(mk2[:], kp1[:], 64.5, None, Alu.is_gt)
            minv = wt([P, M], "minv")
            nc.vector.scalar_tensor_tensor(minv[:], mk2[:], 1.0, notC[:],
                                           Alu.add, Alu.subtract)
            m2 = wt([P, M], "m2")
            nc.vector.scalar_tensor_tensor(m2[:], minv[:], -4000.0, q2[:],
                                           Alu.mult, Alu.add)
            ms = wt([P, M], "ms")
            nc.vector.tensor_tensor_scan(ms[:], m2[:], m2[:], -1e30,
                                         Alu.max, Alu.max)
            rk = wt([P, M], "rk")
            nc.vector.tensor_tensor(rk[:], tk2[:], ms[:], Alu.add)
            nc.vector.tensor_scalar(rk[:], rk[:], 254.0, None, Alu.min)
            nc.vector.scalar_tensor_tensor(rk[:], minv[:], -4000.0, rk[:],
                                           Alu.mult, Alu.add)
            fidx2f = wt([P, 256], "fidx2f")
            fev = fidx2f[:].rearrange("p (a b) -> p a b", b=2)[:, :, 0:1] \
                .rearrange("p a b -> p (a b)")
            fod = fidx2f[:].rearrange("p (a b) -> p a b", b=2)[:, :, 1:2] \
                .rearrange("p a b -> p (a b)")
            nc.scalar.copy(fev, rk[:])
            nc.scalar.activation(fod, rk[:], Act.Identity, bias=ones_c)
            fidx2i = wt([P, 256], "fidx2i", dt.int16)
            nc.gpsimd.tensor_copy(fidx2i[:], fidx2f[:])
            zf2 = wt([P, 256], "zf2", dt.int16)
            nc.gpsimd.local_scatter(zf2[:], nz[:].bitcast(dt.int16),
                                    fidx2i[:], channels=P, num_elems=256,
                                    num_idxs=256)
            zsc = zf2[:].bitcast(dt.float32)

            # ---- fill coarse slots with uniform grid ----
            isCC = wt([P, M], "isCC")
            nc.vector.tensor_scalar(isCC[:], zsc, 0.0, None, Alu.is_equal)
            cum2 = wt([P, M], "cum2")
            nc.vector.tensor_tensor_scan(cum2[:], isCC[:], zero_s[:],
                                         0.0, Alu.add, Alu.add)
            zcf = wt([P, M], "zcf")
            nc.scalar.activation(zcf[:], cum2[:], Act.Identity,
                                 scale=dz_c, bias=aoff_c)
            Z = wt([P, M], "Zm")
            nc.vector.tensor_tensor(Z[:], isCC[:], zcf[:], Alu.mult)
            nc.vector.tensor_tensor(Z[:], Z[:], zsc, Alu.add)
            deltm = wt([P, M], "deltm")
            nc.vector.tensor_tensor(deltm[:, 0:M - 1], Z[:, 1:M],
                                    Z[:, 0:M - 1], Alu.subtract)
            nc.scalar.copy(deltm[:, M - 1:M], sd_c)
            stt['deltm'] = deltm

            # ---- merged xyz -> DRAM bridge (fine MLP input) ----
            xyzm = wt([P, 3, M], "xyzm")
            for c in range(3):
                nc.vector.scalar_tensor_tensor(
                    xyzm[:, c, :], Z[:], rd_s[:, t, c:c + 1],
                    bc(ro_s[:, t, c:c + 1], M), Alu.mult, Alu.add)
            nc.vector.scalar_tensor_tensor(
                xyzm[:].rearrange("p c s -> p (c s)"),
                xyzm[:].rearrange("p c s -> p (c s)"),
                1.0, bc(neg1_c, 3 * M), Alu.min, Alu.max)

            scr2 = dpool.tile([2, 3, 64, M], dt.float32, tag="xyzscr2",
                              name="xyzscr2")
            for hh in range(2):
                nc.gpsimd.dma_start(scr2[hh].rearrange("c pl s -> pl c s"),
                                    xyzm[64 * hh:64 * (hh + 1)])
            stt['scr2'] = scr2

        # ------- phase F: fine MLP + color + composite + image -----------
        def emit_fine(t):
            stt = state[t]
            deltm = stt['deltm']
            scr2 = stt['scr2']
            scr2v = scr2[:].rearrange("h c pl s -> (h c) (pl s)") \
                .bitcast(dt.float32r)

            # per-ray color d-term, laid out for the indicator-matmul fold:
            # dtmT4[a, g*128 + r] = dterm_{r%64}(ray (r//64)*64 + g*16 + a)
            dT_sb = wt([4, P], "dTsb")
            nc.sync.dma_start(dT_sb[:], dT_in[t])
            pDT = ppool.tile([16, 512], dt.float32, tag="pG", name="ppG")
            for g in range(4):
                for hh in range(2):
                    nc.tensor.matmul(
                        pDT[:, g * 128 + hh * 64:g * 128 + (hh + 1) * 64],
                        dT_sb[:, hh * 64 + g * 16:hh * 64 + (g + 1) * 16],
                        dlhs_s[:], start=True, stop=True)
            dtmT4 = wt([16, 512], "dtmT4", dt.float32r)
            nc.vector.tensor_copy(dtmT4[:], pDT[:])

            sgs2 = dpool.tile([2, 64, M], dt.float32, tag="sigscr2",
                              name="sigscr2")
            sgs2f = sgs2[:].rearrange("h p s -> h (p s)")
            rgbs = dpool.tile([2, 3, 64, M], dt.float32, tag="rgbscr",
                              name="rgbscr")
            rgbsw = rgbs[:].rearrange("h c p s -> (h c) (p s)")

            # staged 1024-col pipeline (see coarse); stages: A=L1+relu,
            # B=sigma+wgc(+dterm fold), C=ch1 relu, D=wc2+biased copy
            NGf = 8
            rh1s = [None] * NGf
            ch1s = [None] * NGf
            rhs2k = None
            sg_sb = None
            rgb_sb = None
            for g in range(NGf + 2):
                if g < NGf:
                    gg, h2 = divmod(g, 2)
                    if h2 == 0:
                        rhs2k = mt([6, 2048], "rhs6", dt.float32r)
                        nc.sync.dma_start(
                            rhs2k[:], scr2v[:, gg * 2048:(gg + 1) * 2048])
                    pA = ppool.tile([128, 1024], dt.float32, tag="pA",
                                    name="ppA")
                    for c in range(2):
                        nc.tensor.matmul(
                            pA[:, 512 * c:512 * (c + 1)], lhsT6_s[:],
                            rhs2k[:, h2 * 1024 + 512 * c:
                                  h2 * 1024 + 512 * (c + 1)],
                            start=True, stop=True)
                    rh1g = mt([128, 1024], "rh1", dt.float32r)
                    if g % 2 == 0:
                        nc.vector.tensor_scalar(rh1g[:], pA[:], b1_s[:, 0:1],
                                                0.0, Alu.add, Alu.max)
                    else:
                        nc.scalar.activation(rh1g[:], pA[:], Act.Relu,
                                             bias=b1_s[:, 0:1])
                    rh1s[g] = rh1g
                if 1 <= g <= NGf:
                    G = g - 1
                    gg, h2 = divmod(G, 2)
                    pS = ppool.tile([2, 1024], dt.float32, tag="pA",
                                    name="ppA")
                    for c in range(2):
                        nc.tensor.matmul(pS[:, 512 * c:512 * (c + 1)],
                                         w0p_s[:],
                                         rh1s[G][:, 512 * c:512 * (c + 1)],
                                         start=True, stop=True)
                    if h2 == 0:
                        sg_sb = mt([2, 2048], "sgsb")
                    if G % 2 == 0:
                        nc.vector.tensor_copy(
                            sg_sb[:, h2 * 1024:(h2 + 1) * 1024], pS[:])
                    else:
                        nc.scalar.copy(
                            sg_sb[:, h2 * 1024:(h2 + 1) * 1024], pS[:])
                    if h2 == 1:
                        nc.sync.dma_start(
                            sgs2f[:, gg * 2048:(gg + 1) * 2048], sg_sb[:])
                    # color hidden: relu(wgc.T @ h + dterm); dterm folded in
                    # via an indicator matmul accumulated into the same psum
                    pG = ppool.tile([128, 1024], dt.float32, tag="pG",
                                    name="ppG")
                    for c in range(2):
                        oc = h2 * 2 + c
                        nc.tensor.matmul(pG[:, 512 * c:512 * (c + 1)],
                                         wgc_s[:],
                                         rh1s[G][:, 512 * c:512 * (c + 1)],
                                         start=True, stop=False)
                        nc.tensor.matmul(pG[:, 512 * c:512 * (c + 1)],
                                         dtmT4[:, gg * 128:(gg + 1) * 128],
                                         ind16_s[:, 512 * oc:512 * (oc + 1)],
                                         start=False, stop=True)
                    ch1g = mt([128, 1024], "ch1", dt.float32r)
                    if G % 2 == 0:
                        nc.scalar.activation(ch1g[:], pG[:], Act.Relu)
                    else:
                        nc.vector.tensor_scalar(ch1g[:], pG[:], 0.0, None,
                                                Alu.max)
                    ch1s[G] = ch1g
                if g >= 2:
                    G = g - 2
                    gg, h2 = divmod(G, 2)
                    pC = ppool.tile([6, 1024], dt.float32, tag="pG",
                                    name="ppG")
                    for c in range(2):
                        nc.tensor.matmul(pC[:, 512 * c:512 * (c + 1)],
                                         wc2_s[:],
                                         ch1s[G][:, 512 * c:512 * (c + 1)],
                                         start=True, stop=True)
                    if h2 == 0:
                        rgb_sb = mt([6, 2048], "rgbsb")
                    # psum->sbuf move carries the +bc2 bias (rays side then
                    # computes sigmoid arithmetically -- avoids act-table
                    # swaps between Sigmoid and Relu/Exp)
                    if G % 2 == 0:
                        nc.scalar.activation(
                            rgb_sb[:, h2 * 1024:(h2 + 1) * 1024], pC[:],
                            Act.Identity, bias=bc2_s[:, 0:1])
                    else:
                        nc.vector.tensor_scalar(
                            rgb_sb[:, h2 * 1024:(h2 + 1) * 1024], pC[:],
                            bc2_s[:, 0:1], None, Alu.add)
                    if h2 == 1:
                        nc.sync.dma_start(
                            rgbsw[:, gg * 2048:(gg + 1) * 2048], rgb_sb[:])

            stt['sgs2'] = sgs2
            stt['rgbs'] = rgbs


        # ------- phase X: bridge reload + composite + image --------------
        def emit_comp(t):
            stt = state[t]
            deltm = stt['deltm']
            sgs2 = stt['sgs2']
            rgbs = stt['rgbs']
            # ---- composite in rays layout (telescoped weights) ----
            h20m = wt([P, M], "h20m")
            nc.gpsimd.dma_start(h20m[:], sgs2[:].rearrange("h p s -> (h p) s"))
            rgbp = wt([P, 3, M], "rgbp")
            for hh in range(2):
                nc.gpsimd.dma_start(rgbp[64 * hh:64 * (hh + 1), :, :],
                                    rgbs[hh].rearrange("c p s -> p c s"))

            sigm = wt([P, M], "sigm")
            nc.scalar.activation(sigm[:], h20m[:], Act.Exp, bias=bd2_0c)
            dsg2 = wt([P, M], "dsg2")
            nc.vector.tensor_tensor(dsg2[:], deltm[:], sigm[:], Alu.mult)
            em2 = wt([P, M], "em2")
            nc.scalar.activation(em2[:], dsg2[:], Act.Exp, scale=-DS)
            Tm = wt([P, M], "Tm")
            nc.vector.tensor_tensor_scan(Tm[:], em2[:], zero_s[:], 1.0,
                                         Alu.mult, Alu.add)
            # w_i = T_i - T_{i+1};  Tm_i here is T_{i+1};  bgw = T_M
            wm = wt([P, M], "wm")
            nc.vector.tensor_scalar(wm[:, 0:1], Tm[:, 0:1], -1.0, 1.0,
                                    Alu.mult, Alu.add)
            nc.vector.tensor_tensor(wm[:, 1:M], Tm[:, 0:M - 1], Tm[:, 1:M],
                                    Alu.subtract)
            wmm = wt([P, M], "wmm")
            nc.vector.scalar_tensor_tensor(wmm[:], wm[:], 1e-4, wm[:],
                                           Alu.is_gt, Alu.mult)

            # rgbp holds wc2 pre-activations (+bc2); sigmoid = 1/(1+e^-x)
            erg = wt([P, 3, M], "erg")
            nc.scalar.activation(erg[:].rearrange("p c s -> p (c s)"),
                                 rgbp[:].rearrange("p c s -> p (c s)"),
                                 Act.Exp, scale=-1.0)
            nc.vector.tensor_scalar(erg[:], erg[:], 1.0, None, Alu.add)
            nc.vector.reciprocal(erg[:], erg[:])
            nc.vector.tensor_tensor(
                erg[:], erg[:],
                wmm[:].rearrange("p (o s) -> p o s", o=1)
                .broadcast_to((P, 3, M)), Alu.mult)
            img = wt([P, 3], "img")
            nc.vector.tensor_reduce(img[:], erg[:], AxX, Alu.add)
            nc.vector.scalar_tensor_tensor(img[:], bg_s[:], Tm[:, M - 1:M],
                                           img[:], Alu.mult, Alu.add)
            nc.gpsimd.dma_start(img_out[:, t, :], img[:])

        # ---------------- software-pipelined emission --------------------
        for it in range(T + 3):
            if it < T:
                emit_coarse(it)
            if 2 <= it < T + 2:
                emit_fine(it - 2)
            if it >= 3:
                emit_comp(it - 3)
            if 1 <= it <= T:
                emit_sampling(it - 1)

    nc.compile()
    return nc


def _host_constants(inputs):
    Wd1 = np.asarray(inputs["Wd1"], np.float32)
    bd1 = np.asarray(inputs["bd1"], np.float32)
    Wd2 = np.asarray(inputs["Wd2"], np.float32)
    bd2 = np.asarray(inputs["bd2"], np.float32)
    Wc1 = np.asarray(inputs["Wc1"], np.float32)
    bc1 = np.asarray(inputs["bc1"], np.float32)
    Wc2 = np.asarray(inputs["Wc2"], np.float32)
    bc2 = np.asarray(inputs["bc2"], np.float32)
    tval = float(np.asarray(inputs["time"]).reshape(()))

    W1 = Wd1[:3]
    b1p = bd1 + tval * Wd1[3]
    w0 = Wd2[:, 0:1]
    Wgc = (Wd2[:, 1:].astype(np.float64) @ Wc1[3:].astype(np.float64)) \
        .astype(np.float32)
    bgc = (bd2[1:].astype(np.float64) @ Wc1[3:].astype(np.float64)) \
        .astype(np.float32)
    bd2_0 = float(bd2[0])

    lhsT6 = np.zeros((6, 128), np.float32)
    lhsT6[0:3, 0:64] = W1
    lhsT6[3:6, 64:128] = W1

    b1rep = np.concatenate([b1p, b1p]).reshape(128, 1).astype(np.float32)

    w0pair = np.zeros((128, 32), np.float32)
    w0pair[0:64, 0:1] = w0
    w0pair[64:128, 1:2] = w0

    wgcpair = np.zeros((128, 128), np.float32)
    wgcpair[0:64, 0:64] = Wgc
    wgcpair[64:128, 64:128] = Wgc

    wc2pair = np.zeros((128, 32), np.float32)
    wc2pair[0:64, 0:3] = Wc2
    wc2pair[64:128, 3:6] = Wc2

    dlhs = np.zeros((4, 64), np.float32)
    dlhs[0:3] = Wc1[:3]
    dlhs[3] = bc1 + bgc

    ind16 = np.zeros((16, 2048), np.float32)
    for k in range(16):
        ind16[k, 128 * k:128 * (k + 1)] = 1.0

    v = np.linspace(0.0, 1.0, S, dtype=np.float32)
    return {
        "v128": np.broadcast_to(v, (P, S)).copy(),
        "iotap1_r": np.broadcast_to(np.arange(1, M + 1, dtype=np.float32),
                                    (P, M)).copy(),
        "iotaev62": np.broadcast_to(np.arange(62, dtype=np.float32) * 2,
                                    (P, 62)).copy(),
        "iota62e5": np.broadcast_to(
            np.arange(1, 63, dtype=np.float32) * 1e-5, (P, 62)).copy(),
        "zero128": np.zeros((P, M), np.float32),
        "cc": np.broadcast_to(
            np.array([1.0, -1.0, 1e-9, 0, 0, -1.0 / 128.0,
                      -16777216.0, -2.0, 16777216.0, 0, 0, 0], np.float32),
            (P, 12)).copy(),
        "lhsT6": lhsT6, "b1rep": b1rep, "w0pair": w0pair,
        "wgcpair": wgcpair, "wc2pair": wc2pair, "dlhs": dlhs,
        "ind16": ind16,
        "bgrep": np.broadcast_to(
            np.asarray(inputs["background_color"], np.float32), (P, 3)).copy(),
        "bc2p6": np.concatenate([bc2, bc2]).reshape(6, 1).astype(np.float32),
        "scl": np.broadcast_to(
            np.array([bd2_0, 0, 0, 0], np.float32), (P, 4)).copy(),
    }


def kernel(**inputs):
    global _BUILT
    assert int(inputs["num_steps"]) == S
    assert int(inputs["upsample_steps"]) == U

    if _BUILT is None:
        _BUILT = _build()
    nc = _BUILT

    consts = _host_constants(inputs)
    ro = np.asarray(inputs["rays_o"], np.float32).reshape(NRAYS, 3)
    rd = np.asarray(inputs["rays_d"], np.float32).reshape(NRAYS, 3)

    in_maps = []
    for c in range(NCORES):
        sl_o = ro[c * R:(c + 1) * R].reshape(T, P, 3)
        sl_d = rd[c * R:(c + 1) * R].reshape(T, P, 3)
        dT = np.ones((T, 4, P), np.float32)
        dT[:, 0:3, :] = sl_d.transpose(0, 2, 1)
        m = {
            "rays_o_k": np.ascontiguousarray(sl_o.transpose(1, 0, 2)),
            "rays_d_k": np.ascontiguousarray(sl_d.transpose(1, 0, 2)),
            "dT_k": dT,
        }
        m.update(consts)
        in_maps.append(m)

    res = run_bass_kernel_spmd(nc, in_maps, core_ids=list(range(NCORES)))
    global LAST_RESULT
    LAST_RESULT = res
    outs = []
    for c in range(NCORES):
        img = res.results[c]["img_k"]
        outs.append(img.transpose(1, 0, 2).reshape(R, 3))
    return np.concatenate(outs, 0).reshape(1, NRAYS, 3)
